# revision 1
# baseline (speedup 1.0000x reference)
"""AdaptiveBlockSelfAttention Trainium2 kernel (8-core SPMD, no collectives).

Problem: x[2,256,192,192]; 1x1-conv QKV projections; block-local attention
within 16x16 spatial blocks (8 heads, d=32); output projection + residual;
LayerNorm over channels.

Sharding: the 24 (batch, block-row) slabs are data-parallel -> 3 slabs/core
on 8 cores. Weights replicated. Everything else is core-local.

Layout per 16x16 block ("unit", 256 pixels, C=256):
  - X channel-major [c, p] straight from DRAM.
  - Q^T,K^T channel-major = W^T-stationary matmuls (fp32r, k=128x2).
  - V pixel-major [p, c] = X-stationary matmuls, stored per-head [p, h, 33]
    with a ones column -> the AV matmul emits the softmax denominator row.
  - Per head: S^T[j,i] (k=32), A^T=exp(S^T*scale) on ACT (PSUM->SBUF),
    O^T_u[33,i] = [V|1]^T A^T; 1/l row via DVE reciprocal; gpsimd
    partition_broadcast; normalize+pack into channel-major Oc on DVE.
  - Wo projection pixel-major (Oc-stationary) + residual via PE
    transpose-accumulate of X into the same PSUM group.
  - LayerNorm with bn_stats/bn_aggr (free axis = channels).
  - PE transpose back to channel-major, copy into the output slab.
"""

import numpy as np

import concourse.bacc as bacc
import concourse.tile as tile
import concourse.mybir as mybir
from concourse.bass_utils import run_bass_kernel_spmd
from concourse.masks import make_identity

F32 = mybir.dt.float32
F32R = mybir.dt.float32r
BF16 = mybir.dt.bfloat16
AF = mybir.ActivationFunctionType
ALU = mybir.AluOpType

N_CORES = 8
C = 256
HW_ = 192
BS = 16
NH = 8
D = 32
EPS = 1e-5
SCALE = float(1.0 / np.sqrt(D))

N_SLABS = 3          # block-rows per core
N_UNITS = 12         # 16x16 blocks per block-row


def _build(apply_gb: bool, n_slabs: int = N_SLABS, n_units: int = N_UNITS):
    nc = bacc.Bacc("TRN2", target_bir_lowering=False, debug=False)

    x_ext = nc.declare_dram_parameter("x", [n_slabs, C, BS, HW_], F32R, isOutput=False)
    out_ext = nc.declare_dram_parameter("out", [n_slabs, C, BS, HW_], F32, isOutput=True)
    wq_ext = nc.declare_dram_parameter("wqt", [C, C], F32R, isOutput=False)
    wk_ext = nc.declare_dram_parameter("wkt", [C, C], F32R, isOutput=False)
    wv_ext = nc.declare_dram_parameter("wvt", [C, C], F32R, isOutput=False)
    wo_ext = nc.declare_dram_parameter("wot", [C, C], F32R, isOutput=False)
    gamma_ext = nc.declare_dram_parameter("gamma", [1, C], F32, isOutput=False)
    beta_ext = nc.declare_dram_parameter("beta", [1, C], F32, isOutput=False)

    with tile.TileContext(nc) as tc:
        with (
            tc.tile_pool(name="consts", bufs=1) as consts,
            tc.tile_pool(name="xin", bufs=2) as p_xin,
            tc.tile_pool(name="xu", bufs=3) as p_xu,
            tc.tile_pool(name="xout", bufs=2) as p_xout,
            tc.tile_pool(name="qksb", bufs=2) as p_qk,
            tc.tile_pool(name="vsb", bufs=2) as p_v,
            tc.tile_pool(name="atsb", bufs=4) as p_at,
            tc.tile_pool(name="ocsb", bufs=3) as p_oc,
            tc.tile_pool(name="usb", bufs=2) as p_u,
            tc.tile_pool(name="small", bufs=8) as p_small,
            tc.tile_pool(name="ps_qk", bufs=1, space="PSUM") as ps_qk,
            tc.tile_pool(name="ps_v", bufs=1, space="PSUM") as ps_v,
            tc.tile_pool(name="ps_attn", bufs=4, space="PSUM") as ps_attn,
            tc.tile_pool(name="ps_ep", bufs=1, space="PSUM") as ps_ep,
        ):
            # ---- constants ----
            ident_f32 = consts.tile([128, 128], F32, tag="ident_f32")
            make_identity(nc, ident_f32[:])
            ident = consts.tile([128, 128], F32R)
            nc.vector.tensor_copy(out=ident[:], in_=ident_f32[:])
            ones_c = consts.tile([128, NH], F32, tag="ones_c")
            nc.vector.memset(ones_c[:], 1.0)
            magic_sb = consts.tile([128, 2], mybir.dt.int32, tag="magic")
            nc.vector.memset(magic_sb[:], 0x5F3759DF)

            w_sbs = {}
            for nm, ext in (("wq", wq_ext), ("wk", wk_ext), ("wv", wv_ext), ("wo", wo_ext)):
                w_sb = consts.tile([128, 2, C], F32R, tag=nm)
                nc.sync.dma_start(out=w_sb[:], in_=ext[:].rearrange("(t p) o -> p t o", p=128))
                w_sbs[nm] = w_sb
            wq_sb, wk_sb, wv_sb, wo_sb = (w_sbs[n] for n in ("wq", "wk", "wv", "wo"))
            wo_bf = consts.tile([128, 2, C], BF16, tag="wo_bf")
            nc.vector.tensor_copy(out=wo_bf[:], in_=wo_sb[:].bitcast(F32))
            wv_bf = consts.tile([128, 2, C], BF16, tag="wv_bf")
            nc.vector.tensor_copy(out=wv_bf[:], in_=wv_sb[:].bitcast(F32))

            if apply_gb:
                g_row = consts.tile([1, C], F32, tag="g_row")
                b_row = consts.tile([1, C], F32, tag="b_row")
                nc.sync.dma_start(out=g_row[:], in_=gamma_ext[:])
                nc.sync.dma_start(out=b_row[:], in_=beta_ext[:])
                G128 = consts.tile([128, C], F32, tag="G128")
                B128 = consts.tile([128, C], F32, tag="B128")
                nc.gpsimd.partition_broadcast(out_ap=G128[:], in_ap=g_row[:])
                nc.gpsimd.partition_broadcast(out_ap=B128[:], in_ap=b_row[:])

            for s in range(n_slabs):
                x_sb = p_xin.tile([128, 2, BS, HW_], F32R, tag="x_sb")
                nc.sync.dma_start(
                    out=x_sb[:], in_=x_ext[s].rearrange("(t p) h w -> p t h w", p=128)
                )
                out_sb = p_xout.tile([128, 2, BS, HW_], F32, tag="out_sb")
                if n_units < N_UNITS:
                    nc.vector.memset(out_sb[:], 0.0)

                for j in range(n_units):
                    w0 = BS * j
                    # unit-packed X [c, 2, 256] (pixels contiguous) so matmul
                    # stationary operands have a single free dim
                    xu = p_xu.tile([128, 2, 256], F32R, tag="xu")
                    nc.sync.dma_start(
                        out=xu[:].rearrange("p t (h w) -> p t h w", w=BS),
                        in_=x_sb[:, :, :, w0:w0 + BS],
                    )
                    # ---- Q^T, K^T channel-major ----
                    qk_ps = ps_qk.tile([128, 4, 256], F32, tag="qk")
                    for idx, w_sb in ((0, wq_sb), (2, wk_sb)):
                        for ot in range(2):
                            for kt in range(2):
                                nc.tensor.matmul(
                                    out=qk_ps[:, idx + ot, :],
                                    lhsT=w_sb[:, kt, 128 * ot:128 * ot + 128],
                                    rhs=xu[:, kt, :],
                                    start=(kt == 0), stop=(kt == 1),
                                )
                    # [64, 4, 256]: group g = 2*(hh//2) + ot holds channels
                    # [64*(2*ot + hh//2) ... ) remapped so every head starts at
                    # partition 0 or 32 (matmul base-partition constraint).
                    # [64, 8, 256] bf16: group = half*4 + qk*2 + ot, so every
                    # head's 32 channels start at partition 0 or 32.
                    qkt_sb = p_qk.tile([64, 8, 256], BF16, tag="qkt")
                    nc.scalar.activation(out=qkt_sb[:, 0:4, :], in_=qk_ps[0:64, :, :], func=AF.Copy)
                    nc.scalar.activation(out=qkt_sb[:, 4:8, :], in_=qk_ps[64:128, :, :], func=AF.Copy)

                    xu_bf = p_xu.tile([128, 2, 256], BF16, tag="xu_bf")
                    nc.vector.tensor_copy(out=xu_bf[:], in_=xu[:].bitcast(F32))
                    # ---- V pixel-major with ones column ----
                    v_ps = ps_v.tile([128, 2, 256], F32, tag="v")
                    for pt in range(2):
                        for kt in range(2):
                            nc.tensor.matmul(
                                out=v_ps[:, pt, :],
                                lhsT=xu_bf[:, kt, 128 * pt:128 * pt + 128],
                                rhs=wv_bf[:, kt, :],
                                start=(kt == 0), stop=(kt == 1),
                            )
                    v_sb = p_v.tile([128, 2, NH, D + 2], BF16, tag="v_sb")
                    for pt in range(2):
                        nc.vector.tensor_copy(out=v_sb[:, pt, :, 0:D], in_=v_ps[:, pt, :])
                        nc.vector.tensor_copy(out=v_sb[:, pt, :, D:D + 1], in_=ones_c[:])

                    # ---- per-head attention, software-pipelined: emit ST(h+1)
                    # before AV(h) so the PE isn't idle while ACT runs exp(h) ----
                    oc_sb = p_oc.tile([128, 2, 256], BF16, tag="oc")

                    def emit_st(h):
                        ot, hh = h // 4, h % 4
                        gq = (hh // 2) * 4 + ot
                        gk = (hh // 2) * 4 + 2 + ot
                        off = 32 * (hh % 2)
                        st_ps = ps_attn.tile([128, 2, 256], F32, tag="attn")
                        for jt in range(2):
                            nc.tensor.matmul(
                                out=st_ps[:, jt, :],
                                lhsT=qkt_sb[off:off + D, gk, 128 * jt:128 * jt + 128],
                                rhs=qkt_sb[off:off + D, gq, :],
                                start=True, stop=True,
                            )
                        return st_ps

                    def emit_lchain(h, otu_ps):
                        ot, hh = h // 4, h % 4
                        hp = 32 * hh
                        lrow = p_small.tile([1, 256], F32, tag="lrow")
                        nc.scalar.activation(out=lrow[:], in_=otu_ps[D:D + 1, :], func=AF.Copy)
                        rec = p_small.tile([1, 256], F32, tag="rec")
                        nc.vector.reciprocal_approx_fast(out=rec[:], in_=lrow[:])
                        L32 = p_small.tile([D, 256], F32, tag="L32")
                        nc.gpsimd.partition_broadcast(out_ap=L32[:], in_ap=rec[:])
                        nc.vector.tensor_tensor(
                            out=oc_sb[hp:hp + D, ot, :],
                            in0=otu_ps[0:D, :],
                            in1=L32[:],
                            op=ALU.mult,
                        )

                    st_q = [emit_st(0)]
                    for h in range(NH):
                        if h + 1 < NH:
                            st_q.append(emit_st(h + 1))
                        st_ps = st_q[h]
                        at_sb = p_at.tile([128, 2, 256], BF16, tag="at")
                        nc.scalar.activation(out=at_sb[:], in_=st_ps[:], func=AF.Exp, scale=SCALE)
                        otu_ps = ps_attn.tile([D + 1, 256], F32, tag="attn")
                        for jt in range(2):
                            nc.tensor.matmul(
                                out=otu_ps[:],
                                lhsT=v_sb[:, jt, h, 0:D + 1],
                                rhs=at_sb[:, jt, :],
                                start=(jt == 0), stop=(jt == 1),
                            )
                        emit_lchain(h, otu_ps)

                    # ---- Wo projection (pixel-major) + residual via transpose-accumulate ----
                    pt_ps = ps_ep.tile([128, 2, 256], F32, tag="ep")
                    for pt in range(2):
                        for kt in range(2):
                            nc.tensor.matmul(
                                out=pt_ps[:, pt, :],
                                lhsT=oc_sb[:, kt, 128 * pt:128 * pt + 128],
                                rhs=wo_bf[:, kt, :],
                                start=(kt == 0), stop=False,
                            )
                        for ct in range(2):
                            nc.tensor.matmul(
                                out=pt_ps[:, pt, 128 * ct:128 * ct + 128].bitcast(F32R),
                                lhsT=xu[:, ct, 128 * pt:128 * pt + 128],
                                rhs=ident[:],
                                is_transpose=True, start=False, stop=(ct == 1),
                            )

                    # ---- LayerNorm (free axis = channels) ----
                    u_sb = p_u.tile([128, 2, 256], F32R, tag="u")
                    mv2 = p_small.tile([128, 2, 2], F32, tag="mv2")
                    for pt in range(2):
                        stats = p_small.tile([128, 6], F32, tag="stats")
                        nc.vector.bn_stats(out=stats[:], in_=pt_ps[:, pt, :])
                        nc.vector.bn_aggr(out=mv2[:, pt, :], in_=stats[:])
                    # rstd = 1/sqrt(var+eps) via magic-constant + one Newton step (DVE only)
                    ve = p_small.tile([128, 2], F32, tag="ve")
                    nc.vector.tensor_scalar(out=ve[:], in0=mv2[:, :, 1], scalar1=EPS,
                                            scalar2=None, op0=ALU.add)
                    hbits = p_small.tile([128, 2], mybir.dt.int32, tag="hbits")
                    nc.vector.tensor_scalar(out=hbits[:], in0=ve[:].bitcast(mybir.dt.int32),
                                            scalar1=1, scalar2=None, op0=ALU.arith_shift_right)
                    y0 = p_small.tile([128, 2], F32, tag="y0")
                    nc.vector.tensor_tensor(out=y0[:].bitcast(mybir.dt.int32), in0=magic_sb[:],
                                            in1=hbits[:], op=ALU.subtract)
                    a_t = p_small.tile([128, 2], F32, tag="a_t")
                    nc.vector.tensor_tensor(out=a_t[:], in0=ve[:], in1=y0[:], op=ALU.mult)
                    nc.vector.tensor_tensor(out=a_t[:], in0=a_t[:], in1=y0[:], op=ALU.mult)
                    nc.vector.tensor_scalar(out=a_t[:], in0=a_t[:], scalar1=-0.5, scalar2=1.5,
                                            op0=ALU.mult, op1=ALU.add)
                    rstd2 = p_small.tile([128, 2], F32, tag="rstd2")
                    nc.vector.tensor_tensor(out=rstd2[:], in0=y0[:], in1=a_t[:], op=ALU.mult)
                    # second Newton step for accuracy
                    b_t = p_small.tile([128, 2], F32, tag="b_t")
                    nc.vector.tensor_tensor(out=b_t[:], in0=ve[:], in1=rstd2[:], op=ALU.mult)
                    nc.vector.tensor_tensor(out=b_t[:], in0=b_t[:], in1=rstd2[:], op=ALU.mult)
                    nc.vector.tensor_scalar(out=b_t[:], in0=b_t[:], scalar1=-0.5, scalar2=1.5,
                                            op0=ALU.mult, op1=ALU.add)
                    nc.vector.tensor_tensor(out=rstd2[:], in0=rstd2[:], in1=b_t[:], op=ALU.mult)
                    nmr2 = p_small.tile([128, 2], F32, tag="nmr2")
                    nc.vector.scalar_tensor_tensor(
                        out=nmr2[:], in0=mv2[:, :, 0], scalar=-1.0, in1=rstd2[:],
                        op0=ALU.mult, op1=ALU.mult,
                    )
                    for pt in range(2):
                        nc.vector.tensor_scalar(
                            out=u_sb[:, pt, :], in0=pt_ps[:, pt, :],
                            scalar1=rstd2[:, pt:pt + 1], scalar2=nmr2[:, pt:pt + 1],
                            op0=ALU.mult, op1=ALU.add,
                        )
                        if apply_gb:
                            nc.vector.tensor_tensor(
                                out=u_sb[:, pt, :], in0=u_sb[:, pt, :], in1=G128[:], op=ALU.mult
                            )
                            nc.vector.tensor_tensor(
                                out=u_sb[:, pt, :], in0=u_sb[:, pt, :], in1=B128[:], op=ALU.add
                            )

                    # ---- transpose back to channel-major, stage into out slab ----
                    btp_ps = ps_ep.tile([128, 2, 256], F32, tag="ep")
                    for ct in range(2):
                        for pt in range(2):
                            nc.tensor.matmul(
                                out=btp_ps[:, ct, 128 * pt:128 * pt + 128].bitcast(F32R),
                                lhsT=u_sb[:, pt, 128 * ct:128 * ct + 128],
                                rhs=ident[:],
                                is_transpose=True, start=True, stop=True,
                            )
                    for ct in range(2):
                        nc.vector.tensor_copy(
                            out=out_sb[:, ct, :, w0:w0 + BS], in_=btp_ps[:, ct, :]
                        )

                nc.sync.dma_start(
                    out=out_ext[s].rearrange("(t p) h w -> p t h w", p=128), in_=out_sb[:]
                )

    nc.compile()
    return nc


_CACHE = {}


def _get(apply_gb: bool):
    if apply_gb not in _CACHE:
        _CACHE[apply_gb] = _build(apply_gb)
    return _CACHE[apply_gb]


def _in_maps(x, Wq, Wk, Wv, Wo, gamma, beta):
    x = np.ascontiguousarray(x, dtype=np.float32)
    B = x.shape[0]
    xr = x.reshape(B, C, 12, BS, HW_).transpose(0, 2, 1, 3, 4).reshape(B * 12, C, BS, HW_)
    wqt = np.ascontiguousarray(np.asarray(Wq, dtype=np.float32).T)
    wkt = np.ascontiguousarray(np.asarray(Wk, dtype=np.float32).T)
    wvt = np.ascontiguousarray(np.asarray(Wv, dtype=np.float32).T)
    wot = np.ascontiguousarray(np.asarray(Wo, dtype=np.float32).T)
    g = np.ascontiguousarray(np.asarray(gamma, dtype=np.float32).reshape(1, C))
    b = np.ascontiguousarray(np.asarray(beta, dtype=np.float32).reshape(1, C))
    maps = []
    for core in range(N_CORES):
        maps.append({
            "x": np.ascontiguousarray(xr[core * N_SLABS:(core + 1) * N_SLABS]),
            "wqt": wqt, "wkt": wkt, "wvt": wvt, "wot": wot,
            "gamma": g, "beta": b,
        })
    return maps


def _assemble(results, B=2):
    outs = np.stack([results[i]["out"] for i in range(N_CORES)])
    o = outs.reshape(B, 12, C, BS, HW_).transpose(0, 2, 1, 3, 4).reshape(B, C, 12 * BS, HW_)
    return np.ascontiguousarray(o)


def run(x, Wq, Wk, Wv, Wo, gamma, beta, **spmd_kwargs):
    gamma = np.asarray(gamma, dtype=np.float32)
    beta = np.asarray(beta, dtype=np.float32)
    apply_gb = not (np.allclose(gamma, 1.0) and np.all(beta == 0.0))
    nc = _get(apply_gb)
    maps = _in_maps(x, Wq, Wk, Wv, Wo, gamma, beta)
    res = run_bass_kernel_spmd(nc, maps, core_ids=list(range(N_CORES)), **spmd_kwargs)
    return _assemble(res.results, B=np.asarray(x).shape[0]), res


def kernel(x, Wq, Wk, Wv, Wo, gamma, beta):
    out, _ = run(x, Wq, Wk, Wv, Wo, gamma, beta)
    return out

